# revision 11
# baseline (speedup 1.0000x reference)
"""Gumbel-softmax (hard) attention on 8 Trainium2 NeuronCores.

Reference (per b,h): s = QK^T/sqrt(128); g = -log(-log(u+eps)+eps);
attn = softmax(s+g); out = attn @ V; returns (out, attn).

Sharding: 32 (b,h) pairs -> 4 per core, no communication.

Device layout is fully transposed ([k partitions, q free]) so that
softmax sums go through the PE (ones-matmul), the attn@V contraction
uses V in natural layout as lhsT, and no on-device transposes exist:
  inputs : qtn = -Q^T/sqrt(128), kt = K^T, v (natural), ut = noise^T
  outputs: attnt = attn^T, outt = out^T      (host transposes on gather)

Per (pair, q-chunk of 512):
  u = ln(u + eps); u = ln(eps - u)                       # ACT, in place -> l2 (= -gumbel)
  for kt16: psum_s = K_kt @ qtn_chunk (= -s^T, fp32)     # PE
            u[:,kt,:] += psum_s (= l2 - s)               # DVE
  num = exp(-u - SHIFT) -> float32r                      # ACT (= e^{s+g-SHIFT})
  for kt16: Z[1,512]  += ones^T @ num_kt                 # PE fp32r
            oT[128,512] += v_kt^T @ num_kt               # PE fp32r
  invz = 1/Z                                             # DVE
  zb = ones1 x invz (broadcast to 128 partitions)        # PE
  attnt_chunk = num * zb                                 # GPSIMD, in place
  outt_chunk  = oT * zb                                  # DVE
"""

import math

import numpy as np

N_CORES = 8
B, H, S, D = 2, 16, 2048, 128
PAIRS = (B * H) // N_CORES  # 4
KT = S // 128               # 16 k-tiles
QC = 4                      # q chunks
QW = S // QC                # 512
EPS = 1e-10
SHIFT = 25.0

_CACHE = {}


def _build():
    import concourse.bacc as bacc
    import concourse.tile as tile
    from concourse import mybir

    f32 = mybir.dt.float32
    f32r = mybir.dt.float32r
    AF = mybir.ActivationFunctionType

    nc = bacc.Bacc("TRN2", target_bir_lowering=False, debug=False,
                   num_devices=N_CORES)
    qtn = nc.dram_tensor("qtn", [PAIRS, 128, S], f32, kind="ExternalInput").ap()
    kt = nc.dram_tensor("kt", [PAIRS, 128, S], f32, kind="ExternalInput").ap()
    v = nc.dram_tensor("v", [PAIRS, S, 128], f32, kind="ExternalInput").ap()
    ut = nc.dram_tensor("ut", [PAIRS, S, S], f32, kind="ExternalInput").ap()
    attnt = nc.dram_tensor("attnt", [PAIRS, S, S], f32, kind="ExternalOutput").ap()
    outt = nc.dram_tensor("outt", [PAIRS, 128, S], f32, kind="ExternalOutput").ap()

    with tile.TileContext(nc) as tc:
        with tc.tile_pool(name="const", bufs=1) as constp, \
             tc.tile_pool(name="pairp", bufs=1) as pairp, \
             tc.tile_pool(name="up", bufs=2) as up, \
             tc.tile_pool(name="nump", bufs=2) as nump, \
             tc.tile_pool(name="smallp", bufs=2) as smallp, \
             tc.tile_pool(name="outp", bufs=2) as outp, \
             tc.tile_pool(name="ps_s", bufs=2, space="PSUM") as ps_sp, \
             tc.tile_pool(name="ps_z", bufs=2, space="PSUM") as ps_zp, \
             tc.tile_pool(name="ps_o", bufs=2, space="PSUM") as ps_op, \
             tc.tile_pool(name="ps_b", bufs=2, space="PSUM") as ps_bp:

            ones_f = constp.tile([128, 1], f32)
            nc.vector.memset(ones_f, 1.0)
            ones_r = constp.tile([128, 1], f32r)
            nc.vector.tensor_copy(ones_r, ones_f)
            ones1 = constp.tile([1, 128], f32)
            nc.vector.memset(ones1, 1.0)
            eps_b = constp.tile([128, 1], f32)
            nc.vector.memset(eps_b, EPS)
            nshift_b = constp.tile([128, 1], f32)
            nc.vector.memset(nshift_b, -SHIFT)

            for p in range(PAIRS):
                qtn_sb = pairp.tile([128, S], f32, tag="qtn")
                kt_sb = pairp.tile([128, S], f32, tag="kt")
                v_sb = pairp.tile([128, KT, 128], f32, tag="v")
                vr_sb = pairp.tile([128, KT, 128], f32r, tag="vr")
                nc.sync.dma_start(out=qtn_sb, in_=qtn[p])
                nc.sync.dma_start(out=kt_sb, in_=kt[p])
                nc.sync.dma_start(
                    out=v_sb, in_=v[p].rearrange("(t p) d -> p t d", p=128))
                nc.vector.tensor_copy(vr_sb, v_sb)

                ut_p = ut[p].rearrange("(t p) q -> p t q", p=128)
                attnt_p = attnt[p].rearrange("(t p) q -> p t q", p=128)
                outt_sb = outp.tile([128, S], f32, tag="outt")

                for qc in range(QC):
                    qs = qc * QW
                    u_sb = up.tile([128, KT, QW], f32, tag="u")
                    nc.sync.dma_start(out=u_sb, in_=ut_p[:, :, qs:qs + QW])
                    # l1 = ln(u + eps); l2 = ln(eps - l1)   (gumbel = -l2)
                    nc.scalar.activation(u_sb, u_sb, AF.Ln, bias=eps_b, scale=1.0)
                    nc.scalar.activation(u_sb, u_sb, AF.Ln, bias=eps_b, scale=-1.0)

                    for t in range(KT):
                        ps_s = ps_sp.tile([128, QW], f32, tag="s")
                        nc.tensor.matmul(
                            ps_s, kt_sb[:, t * 128:(t + 1) * 128],
                            qtn_sb[:, qs:qs + QW], start=True, stop=True)
                        nc.vector.tensor_add(u_sb[:, t, :], u_sb[:, t, :], ps_s)

                    # num's only writer is this exp (fp32r out), so the
                    # fp32r matmuls below pass the BIR rounding check
                    num_sb = nump.tile([128, KT, QW], f32r, tag="num")
                    nc.scalar.activation(num_sb, u_sb, AF.Exp,
                                         bias=nshift_b, scale=-1.0)

                    ps_z = ps_zp.tile([1, QW], f32, tag="z")
                    ps_o = ps_op.tile([128, QW], f32, tag="o")
                    for t in range(KT):
                        nc.tensor.matmul(ps_z, ones_r, num_sb[:, t, :],
                                         start=(t == 0), stop=(t == KT - 1))
                        nc.tensor.matmul(ps_o, vr_sb[:, t, :], num_sb[:, t, :],
                                         start=(t == 0), stop=(t == KT - 1))

                    z_sb = smallp.tile([1, QW], f32, tag="zc")
                    nc.vector.tensor_copy(z_sb, ps_z)
                    invz = smallp.tile([1, QW], f32, tag="iz")
                    nc.vector.reciprocal(invz, z_sb)
                    ps_b = ps_bp.tile([128, QW], f32, tag="b")
                    nc.tensor.matmul(ps_b, ones1, invz, start=True, stop=True)
                    zb_sb = smallp.tile([128, QW], f32, tag="zb")
                    nc.vector.tensor_copy(zb_sb, ps_b)

                    # normalized attn lands in u_sb (free after the exp)
                    nc.gpsimd.tensor_mul(
                        u_sb, num_sb.bitcast(f32),
                        zb_sb.unsqueeze(1).broadcast_to([128, KT, QW]))
                    nc.sync.dma_start(out=attnt_p[:, :, qs:qs + QW], in_=u_sb)
                    nc.vector.tensor_mul(outt_sb[:, qs:qs + QW], ps_o, zb_sb)

                nc.sync.dma_start(out=outt[p], in_=outt_sb)

    nc.compile()
    return nc


def _get_nc():
    if "nc" not in _CACHE:
        _CACHE["nc"] = _build()
    return _CACHE["nc"]


def make_in_maps(query, key, value, uniform_noise):
    scale = -1.0 / math.sqrt(float(D))
    qtn = np.ascontiguousarray(
        query.transpose(0, 1, 3, 2)).reshape(B * H, D, S) * np.float32(scale)
    kt = np.ascontiguousarray(key.transpose(0, 1, 3, 2)).reshape(B * H, D, S)
    v = np.ascontiguousarray(value).reshape(B * H, S, D)
    ut = np.ascontiguousarray(
        uniform_noise.transpose(0, 1, 3, 2)).reshape(B * H, S, S)
    in_maps = []
    for c in range(N_CORES):
        sl = slice(c * PAIRS, (c + 1) * PAIRS)
        in_maps.append({
            "qtn": np.ascontiguousarray(qtn[sl]),
            "kt": kt[sl], "v": v[sl], "ut": ut[sl],
        })
    return in_maps


def assemble(results):
    out = np.empty((B * H, S, D), np.float32)
    attn = np.empty((B * H, S, S), np.float32)
    for c in range(N_CORES):
        for i in range(PAIRS):
            out[c * PAIRS + i] = results[c]["outt"][i].T
            attn[c * PAIRS + i] = results[c]["attnt"][i].T
    return (out.reshape(B, H, S, D), attn.reshape(B, H, S, S))


def kernel(query, key, value, uniform_noise, mask=None, **_):
    from concourse.bass_utils import run_bass_kernel_spmd

    nc = _get_nc()
    in_maps = make_in_maps(
        np.asarray(query, np.float32), np.asarray(key, np.float32),
        np.asarray(value, np.float32), np.asarray(uniform_noise, np.float32))
    res = run_bass_kernel_spmd(nc, in_maps, core_ids=list(range(N_CORES)))
    return assemble(res.results)


# revision 20
# speedup vs baseline: 1.0608x; 1.0608x over previous
"""Gumbel-softmax (hard) attention on 8 Trainium2 NeuronCores.

Reference (per b,h): s = QK^T/sqrt(128); g = -log(-log(u+eps)+eps);
attn = softmax(s+g); out = attn @ V; returns (out, attn).

Sharding: 32 (b,h) pairs -> 4 per core, no communication.

Device layout is fully transposed ([k partitions, q free]) so that
softmax sums go through the PE (ones-matmul), the attn@V contraction
uses V in natural layout as lhsT, and no on-device transposes exist:
  inputs : qtn = -Q^T/sqrt(128), kt = K^T, v (natural), ut = noise^T
  outputs: attnt = attn^T, outt = out^T      (host transposes on gather)

Per (pair, q-chunk of 512):
  u = ln(u + eps); u = ln(eps - u)                       # ACT, in place -> l2 (= -gumbel)
  for kt16: psum_s = K_kt @ qtn_chunk (= -s^T, fp32)     # PE
            u[:,kt,:] += psum_s (= l2 - s)               # DVE
  num = exp(-u - SHIFT) -> float32r                      # ACT (= e^{s+g-SHIFT})
  for kt16: Z[1,512]  += ones^T @ num_kt                 # PE fp32r
            oT[128,512] += v_kt^T @ num_kt               # PE fp32r
  invz = 1/Z                                             # DVE
  zb = ones1 x invz (broadcast to 128 partitions)        # PE
  attnt_chunk = num * zb                                 # GPSIMD, in place
  outt_chunk  = oT * zb                                  # DVE
"""

import math

import numpy as np

N_CORES = 8
B, H, S, D = 2, 16, 2048, 128
PAIRS = (B * H) // N_CORES  # 4
KT = S // 128               # 16 k-tiles
QC = 4                      # q chunks
QW = S // QC                # 512
EPS = 1e-10
SHIFT = 25.0

_CACHE = {}


def _build(nrep=1):
    import concourse.bacc as bacc
    import concourse.tile as tile
    from concourse import mybir

    f32 = mybir.dt.float32
    f32r = mybir.dt.float32r
    AF = mybir.ActivationFunctionType

    nc = bacc.Bacc("TRN2", target_bir_lowering=False, debug=False,
                   num_devices=N_CORES)
    qtn = nc.dram_tensor("qtn", [PAIRS, 128, S], f32, kind="ExternalInput").ap()
    kt = nc.dram_tensor("kt", [PAIRS, 128, S], f32, kind="ExternalInput").ap()
    v = nc.dram_tensor("v", [PAIRS, S, 128], f32, kind="ExternalInput").ap()
    ut = nc.dram_tensor("ut", [PAIRS, S, S], f32, kind="ExternalInput").ap()
    attnt = nc.dram_tensor("attnt", [PAIRS, S, S], f32, kind="ExternalOutput").ap()
    outt = nc.dram_tensor("outt", [PAIRS, 128, S], f32, kind="ExternalOutput").ap()

    with tile.TileContext(nc) as tc:
        with tc.tile_pool(name="const", bufs=1) as constp, \
             tc.tile_pool(name="pairp", bufs=1) as pairp, \
             tc.tile_pool(name="up", bufs=2) as up, \
             tc.tile_pool(name="l2rp", bufs=1) as l2rp, \
             tc.tile_pool(name="nump", bufs=1) as nump, \
             tc.tile_pool(name="smallp", bufs=2) as smallp, \
             tc.tile_pool(name="outp", bufs=2) as outp, \
             tc.tile_pool(name="ps_s", bufs=2, space="PSUM") as ps_sp, \
             tc.tile_pool(name="ps_z", bufs=2, space="PSUM") as ps_zp, \
             tc.tile_pool(name="ps_o", bufs=2, space="PSUM") as ps_op, \
             tc.tile_pool(name="ps_b", bufs=2, space="PSUM") as ps_bp:

            ones_f = constp.tile([128, 1], f32)
            nc.vector.memset(ones_f, 1.0)
            ones_r = constp.tile([128, 1], f32r)
            nc.vector.tensor_copy(ones_r, ones_f)
            # fp32r identity for PSUM-side gumbel add (I.T @ l2)
            from concourse.masks import make_identity
            ident_f = constp.tile([128, 128], f32)
            make_identity(nc, ident_f)
            ident_r = constp.tile([128, 128], f32r)
            nc.vector.tensor_copy(ident_r, ident_f)
            ones1 = constp.tile([1, 128], f32)
            nc.vector.memset(ones1, 1.0)
            eps_b = constp.tile([128, 1], f32)
            nc.vector.memset(eps_b, EPS)
            nshift_b = constp.tile([128, 1], f32)
            nc.vector.memset(nshift_b, -SHIFT)

            for p in [pp for _ in range(nrep) for pp in range(PAIRS)]:
                qtn_sb = pairp.tile([128, S], f32, tag="qtn")
                kt_sb = pairp.tile([128, S], f32, tag="kt")
                v_sb = pairp.tile([128, KT, 128], f32, tag="v")
                vr_sb = pairp.tile([128, KT, 128], f32r, tag="vr")
                nc.sync.dma_start(out=qtn_sb, in_=qtn[p])
                nc.sync.dma_start(out=kt_sb, in_=kt[p])
                nc.sync.dma_start(
                    out=v_sb, in_=v[p].rearrange("(t p) d -> p t d", p=128))
                nc.vector.tensor_copy(vr_sb, v_sb)

                ut_p = ut[p].rearrange("(t p) q -> p t q", p=128)
                attnt_p = attnt[p].rearrange("(t p) q -> p t q", p=128)
                outt_sb = outp.tile([128, S], f32, tag="outt")

                for qc in range(QC):
                    qs = qc * QW
                    u_sb = up.tile([128, KT, QW], f32, tag="u")
                    nc.sync.dma_start(out=u_sb, in_=ut_p[:, :, qs:qs + QW])
                    # l1 = ln(u + eps); l2 = ln(eps - l1)   (gumbel = -l2)
                    nc.scalar.activation(u_sb, u_sb, AF.Ln, bias=eps_b, scale=1.0)
                    l2r = l2rp.tile([128, KT, QW], f32r, tag="l2")
                    nc.scalar.activation(l2r, u_sb, AF.Ln, bias=eps_b, scale=-1.0)

                    num_sb = nump.tile([128, KT, QW], f32r, tag="num")
                    for t in range(KT):
                        # psum = -s^T (fp32) then += l2 via fp32r I.T @ l2
                        ps_s = ps_sp.tile([128, QW], f32, tag="s")
                        nc.tensor.matmul(
                            ps_s, kt_sb[:, t * 128:(t + 1) * 128],
                            qtn_sb[:, qs:qs + QW], start=True, stop=False)
                        nc.tensor.matmul(ps_s, ident_r, l2r[:, t, :],
                                         start=False, stop=True)
                        # num = exp(s - l2 - SHIFT), straight from PSUM
                        nc.scalar.activation(num_sb[:, t, :], ps_s, AF.Exp,
                                             bias=nshift_b, scale=-1.0)

                    ps_z = ps_zp.tile([1, QW], f32, tag="z")
                    ps_o = ps_op.tile([128, QW], f32, tag="o")
                    for t in range(KT):
                        nc.tensor.matmul(ps_z, ones_r, num_sb[:, t, :],
                                         start=(t == 0), stop=(t == KT - 1))
                        nc.tensor.matmul(ps_o, vr_sb[:, t, :], num_sb[:, t, :],
                                         start=(t == 0), stop=(t == KT - 1))

                    z_sb = smallp.tile([1, QW], f32, tag="zc")
                    nc.vector.tensor_copy(z_sb, ps_z)
                    invz = smallp.tile([1, QW], f32, tag="iz")
                    nc.vector.reciprocal(invz, z_sb)
                    ps_b = ps_bp.tile([128, QW], f32, tag="b")
                    nc.tensor.matmul(ps_b, ones1, invz, start=True, stop=True)
                    zb_sb = smallp.tile([128, QW], f32, tag="zb")
                    nc.vector.tensor_copy(zb_sb, ps_b)

                    # normalized attn lands in u_sb (free after log2)
                    for t in range(KT):
                        nc.gpsimd.tensor_mul(u_sb[:, t, :],
                                             num_sb[:, t, :].bitcast(f32), zb_sb)
                    nc.sync.dma_start(out=attnt_p[:, :, qs:qs + QW], in_=u_sb)
                    nc.vector.tensor_mul(outt_sb[:, qs:qs + QW], ps_o, zb_sb)

                nc.sync.dma_start(out=outt[p], in_=outt_sb)

    nc.compile()
    return nc


def _get_nc():
    if "nc" not in _CACHE:
        _CACHE["nc"] = _build()
    return _CACHE["nc"]


def make_in_maps(query, key, value, uniform_noise):
    scale = -1.0 / math.sqrt(float(D))
    qtn = np.ascontiguousarray(
        query.transpose(0, 1, 3, 2)).reshape(B * H, D, S) * np.float32(scale)
    kt = np.ascontiguousarray(key.transpose(0, 1, 3, 2)).reshape(B * H, D, S)
    v = np.ascontiguousarray(value).reshape(B * H, S, D)
    ut = np.ascontiguousarray(
        uniform_noise.transpose(0, 1, 3, 2)).reshape(B * H, S, S)
    in_maps = []
    for c in range(N_CORES):
        sl = slice(c * PAIRS, (c + 1) * PAIRS)
        in_maps.append({
            "qtn": np.ascontiguousarray(qtn[sl]),
            "kt": kt[sl], "v": v[sl], "ut": ut[sl],
        })
    return in_maps


def assemble(results):
    out = np.empty((B * H, S, D), np.float32)
    attn = np.empty((B * H, S, S), np.float32)
    for c in range(N_CORES):
        for i in range(PAIRS):
            out[c * PAIRS + i] = results[c]["outt"][i].T
            attn[c * PAIRS + i] = results[c]["attnt"][i].T
    return (out.reshape(B, H, S, D), attn.reshape(B, H, S, S))


def kernel(query, key, value, uniform_noise, mask=None, **_):
    from concourse.bass_utils import run_bass_kernel_spmd

    nc = _get_nc()
    in_maps = make_in_maps(
        np.asarray(query, np.float32), np.asarray(key, np.float32),
        np.asarray(value, np.float32), np.asarray(uniform_noise, np.float32))
    res = run_bass_kernel_spmd(nc, in_maps, core_ids=list(range(N_CORES)))
    return assemble(res.results)
